# revision 24
# baseline (speedup 1.0000x reference)
"""DiceLoss kernel for Trainium2 (8 NeuronCores, SPMD spatial sharding).

x: (4, 8, 64, 256, 256) f32 logits; target: (4, 1, 64, 256, 256) int labels.
loss = 1 - mean_b mean_c (2*inter[b,c]+1)/(psum[b,c]+tsum[b,c]+1)
  with p = softmax(x, axis=1) flattened over spatial S:
  inter[b,c] = sum_s p[b,c,s]*[t==c], psum = sum_s p, tsum = count(t==c).

Sharding: each core takes a contiguous 1/8 slice of S for all (b, c) and
produces partial sums of (psum, inter, tsum); the host reduces across
cores in f64 and applies the dice formula.

Per-core device pipeline, chunked [128 part, 8 class, 1024 free],
engine-balanced from measured per-op rates:
  SWDGE DMA: x f32->f16 cast, target int->f16 cast
  ScalarE: E = exp(x) fp16; lse = ln(denom); r = exp(-lse) fp16
  TensorE: denom via identity-matmul PSUM accumulation (contiguous rhs)
  VectorE: u_c = (E_c+0)*r scalar_tensor_tensor with accum_out -> psum
           m_c = (T==c) tensor_scalar (4x mode)
           w8 = u8*m8 one broadcast-free tensor_tensor (2x mode)
  TensorE: inter = ones32-matmuls over w8 slices, per-class [32-row, 512]
           PSUM regions via out base_partition (col-group placement)
  tsum: classes 0-3 on TensorE (same trick), 4-7 on ScalarE Copy+accum
"""

import sys
import time

import numpy as np

for _p in ("/opt/trn_rl_repo",):
    if _p not in sys.path:
        sys.path.insert(0, _p)

B = 4
C = 8
S = 64 * 256 * 256  # 4,194,304 spatial positions per (b, c)
NCORES = 8
SC = S // NCORES    # 524,288 positions per core per b
P = 128
F = 1024            # free positions per partition per chunk
CHUNK = P * F       # 131,072 positions per chunk
KCH = SC // CHUNK   # 4 chunks per b per core
SMOOTH = 1.0

PROFILE = False
RUN_KWARGS = {}
LAST = {}

_cache = {}

# walrus ldw-opt elides redundant LDWEIGHTS for repeated stationaries
# (concourse pins it off; our matmul streams reuse one stationary per pass)
# (walrus rejects bacc's explicit InstLdweights under ldw-opt; keep False)
ENABLE_LDW_OPT = False


def _patch_ldw_opt():
    import concourse.bass_utils as bu

    orig = bu.run_command

    def patched(argv, **kw):
        argv = ["--enable-ldw-opt=true" if a == "--enable-ldw-opt=false"
                else a for a in argv]
        return orig(argv, **kw)

    bu.run_command = patched
    return lambda: setattr(bu, "run_command", orig)


def _pin_act_tables():
    """Make natural_log_exp_and_others the only table providing Exp/Ln so
    the table-load pass emits one load instead of thrashing between the
    exp-only and ln-only sets. List positions are preserved (walrus maps
    sets by index)."""
    import concourse.bacc as bacc_mod
    from concourse import mybir

    orig = bacc_mod.get_activation_tables

    def patched(arch):
        tables = dict(orig(arch))
        exp = mybir.ActivationFunctionType.Exp
        ln = mybir.ActivationFunctionType.Ln
        for name, funcs in tables.items():
            if name != "natural_log_exp_and_others" and (
                exp in funcs or ln in funcs
            ):
                tables[name] = funcs - {exp, ln}
        return tables

    bacc_mod.get_activation_tables = patched
    return lambda: setattr(bacc_mod, "get_activation_tables", orig)


def _build(tgt_words):
    """Build + compile the Bass program. tgt_words=2 for int64 targets
    (int32 lo/hi pairs), 1 for int32 targets."""
    import concourse.bacc as bacc
    import concourse.tile as tile
    from concourse import mybir

    f32 = mybir.dt.float32
    f16 = mybir.dt.float16
    i32 = mybir.dt.int32
    Alu = mybir.AluOpType
    Act = mybir.ActivationFunctionType

    restore = _pin_act_tables()
    try:
        nc = bacc.Bacc("TRN2", target_bir_lowering=False)

        x_in = nc.dram_tensor("x", [B, C, SC], f32, kind="ExternalInput")
        if tgt_words == 2:
            t_in = nc.dram_tensor("t", [B, SC, 2], i32, kind="ExternalInput")
        else:
            t_in = nc.dram_tensor("t", [B, SC], i32, kind="ExternalInput")
        # psum partials: per-partition accum slots per (class, chunk)
        o_ps = nc.dram_tensor("o_ps", [B, P, C * KCH], f32,
                              kind="ExternalOutput")
        # inter partials: 2 PSUM banks, class c at row 32*(c%4) of bank c//4
        o_in = nc.dram_tensor("o_in", [B, 2, P, 512], f32,
                              kind="ExternalOutput")
        # tsum partials: classes 0-3 PE bank rows; 4-7 ScalarE accum slots
        o_tp = nc.dram_tensor("o_tp", [B, P, 512], f32, kind="ExternalOutput")
        o_ta = nc.dram_tensor("o_ta", [B, P, 4 * KCH], f32,
                              kind="ExternalOutput")

        xv = x_in[:].rearrange("b c (k p f) -> b k p c f", p=P, f=F)
        if tgt_words == 2:
            tv = t_in[:].rearrange("b (k p f) w -> b k p f w", p=P, f=F)
        else:
            tv = t_in[:].rearrange("b (k p f) -> b k p f", p=P, f=F)

        with tile.TileContext(nc) as tc:
            with (
                tc.tile_pool(name="const", bufs=1) as cpool,
                tc.tile_pool(name="xbuf", bufs=2) as xpool,
                tc.tile_pool(name="ebuf", bufs=2) as epool,
                tc.tile_pool(name="ubuf", bufs=2) as upool,
                tc.tile_pool(name="mbuf", bufs=2) as mpool,
                tc.tile_pool(name="wbuf", bufs=2) as wpool,
                tc.tile_pool(name="small", bufs=2) as spool,
                tc.tile_pool(name="stats", bufs=2) as stpool,
                tc.tile_pool(name="psum", bufs=2, space="PSUM") as ppool,
                tc.tile_pool(name="acc", bufs=1, space="PSUM") as apool,
            ):
                ident_i = cpool.tile([P, P], i32)
                nc.gpsimd.iota(ident_i[:], [[1, P]], channel_multiplier=-1)
                ident = cpool.tile([P, P], f16)
                nc.vector.tensor_scalar(
                    out=ident[:], in0=ident_i[:], scalar1=0, scalar2=None,
                    op0=Alu.is_equal,
                )
                ones32 = cpool.tile([P, 32], f16)
                nc.vector.memset(ones32[:], 1.0)

                for b in range(B):
                    st_ps = stpool.tile([P, C * KCH], f32, tag="st_ps")
                    st_ta = stpool.tile([P, 4 * KCH], f32, tag="st_ta")
                    acc_in0 = apool.tile([P, 512], f32, tag="acc_in0")
                    acc_in1 = apool.tile([P, 512], f32, tag="acc_in1")
                    acc_in = [acc_in0, acc_in1]
                    acc_tp = apool.tile([P, 512], f32, tag="acc_tp")

                    for kk in range(KCH):
                        first = kk == 0
                        last = kk == KCH - 1

                        xt = xpool.tile([P, C, F], f16, tag="xt")
                        nc.gpsimd.dma_start(out=xt[:], in_=xv[b, kk])

                        t16 = spool.tile([P, F], f16, tag="t16")
                        if tgt_words == 2:
                            nc.gpsimd.dma_start(out=t16[:],
                                                in_=tv[b, kk, :, :, 0])
                        else:
                            nc.gpsimd.dma_start(out=t16[:], in_=tv[b, kk])

                        e16 = epool.tile([P, C, F], f16, tag="e16")
                        nc.scalar.activation(e16[:], xt[:], Act.Exp)

                        dps = ppool.tile([P, F], f32, tag="dps")
                        for h in range(F // 512):
                            sl = slice(h * 512, (h + 1) * 512)
                            for c in range(C):
                                nc.tensor.matmul(
                                    dps[:, sl], ident[:], e16[:, c, sl],
                                    start=(c == 0), stop=(c == C - 1),
                                )

                        lse = spool.tile([P, F], f32, tag="lse")
                        nc.scalar.activation(lse[:], dps[:], Act.Ln)
                        r16 = spool.tile([P, F], f16, tag="r16")
                        nc.scalar.activation(r16[:], lse[:], Act.Exp,
                                             scale=-1.0)

                        # u_c = (E_c + 0) * r, accum -> psum partials
                        u8 = upool.tile([P, C, F], f16, tag="u8")
                        for c in range(C):
                            col = c * KCH + kk
                            nc.vector.scalar_tensor_tensor(
                                out=u8[:, c], in0=e16[:, c], scalar=0.0,
                                in1=r16[:], op0=Alu.add, op1=Alu.mult,
                                accum_out=st_ps[:, col:col + 1],
                            )
                        # masks (4x tensor_scalar)
                        m8 = mpool.tile([P, C, F], f16, tag="m8")
                        for c in range(C):
                            nc.vector.tensor_scalar(
                                out=m8[:, c], in0=t16[:], scalar1=float(c),
                                scalar2=None, op0=Alu.is_equal,
                            )
                        # w = u * m in one 8-class op (2x tensor_tensor)
                        w8 = wpool.tile([P, C, F], f16, tag="w8")
                        nc.vector.tensor_tensor(
                            out=w8[:], in0=u8[:], in1=m8[:], op=Alu.mult)

                        # inter-red: ones32 matmuls, class c -> rows
                        # 32*(c%4).. of bank c//4
                        for c in range(C):
                            row = 32 * (c % 4)
                            tp = (0, row) if row == 96 else None
                            for h in range(F // 512):
                                sl = slice(h * 512, (h + 1) * 512)
                                nc.tensor.matmul(
                                    acc_in[c // 4][row:row + 32, :],
                                    ones32[:], w8[:, c, sl],
                                    start=(first and h == 0),
                                    stop=(last and h == F // 512 - 1),
                                    tile_position=tp,
                                    skip_group_check=True,
                                )
                        # tsum-red classes 0-3 on PE
                        for c in range(4):
                            row = 32 * c
                            tp = (0, row) if row == 96 else None
                            for h in range(F // 512):
                                sl = slice(h * 512, (h + 1) * 512)
                                nc.tensor.matmul(
                                    acc_tp[row:row + 32, :],
                                    ones32[:], m8[:, c, sl],
                                    start=(first and h == 0),
                                    stop=(last and h == F // 512 - 1),
                                    tile_position=tp,
                                    skip_group_check=True,
                                )
                        # tsum-red classes 4-7 on ScalarE (Copy + accum)
                        for c in range(4, C):
                            col = (c - 4) * KCH + kk
                            dump = spool.tile([P, F], f16, tag="dump")
                            nc.scalar.activation(
                                dump[:], m8[:, c], Act.Copy,
                                accum_out=st_ta[:, col:col + 1],
                            )

                    # flush per-b results
                    nc.sync.dma_start(out=o_ps[b], in_=st_ps[:])
                    nc.sync.dma_start(out=o_ta[b], in_=st_ta[:])
                    for half in range(2):
                        stg = spool.tile([P, 512], f32, tag="stg")
                        nc.vector.tensor_copy(stg[:], acc_in[half][:])
                        nc.sync.dma_start(out=o_in[b, half], in_=stg[:])
                    stg_tp = spool.tile([P, 512], f32, tag="stg")
                    nc.vector.tensor_copy(stg_tp[:], acc_tp[:])
                    nc.sync.dma_start(out=o_tp[b], in_=stg_tp[:])

        nc.compile()
    finally:
        restore()
    return nc


def kernel(x, target):
    x = np.asarray(x)
    target = np.asarray(target)
    assert x.shape == (B, C, 64, 256, 256) and x.dtype == np.float32
    tgt_words = 2 if target.dtype == np.int64 else 1

    if tgt_words not in _cache:
        _cache[tgt_words] = _build(tgt_words)
    nc = _cache[tgt_words]

    xr = x.reshape(B, C, S)
    tr = target.reshape(B, S)

    in_maps = []
    for i in range(NCORES):
        sl = slice(i * SC, (i + 1) * SC)
        xs = np.ascontiguousarray(xr[:, :, sl])
        ts = np.ascontiguousarray(tr[:, sl])
        if tgt_words == 2:
            ts = ts.view(np.int32).reshape(B, SC, 2)
        else:
            ts = ts.astype(np.int32, copy=False)
        in_maps.append({"x": xs, "t": ts})

    from concourse.bass_utils import run_bass_kernel_spmd

    restore_ldw = _patch_ldw_opt() if ENABLE_LDW_OPT else (lambda: None)
    t0 = time.perf_counter()
    try:
        res = run_bass_kernel_spmd(
            nc, in_maps, list(range(NCORES)), trace=PROFILE, **RUN_KWARGS,
        )
    finally:
        restore_ldw()
    t1 = time.perf_counter()
    LAST["wall_s"] = t1 - t0
    LAST["exec_time_ns"] = res.exec_time_ns
    LAST["mean_exec_time_ns"] = res.mean_exec_time_ns

    ps = np.zeros((B, C), np.float64)
    it = np.zeros((B, C), np.float64)
    tsm = np.zeros((B, C), np.float64)
    for r in res.results:
        ps += (r["o_ps"].astype(np.float64)
               .reshape(B, P, C, KCH).sum(axis=(1, 3)))
        # inter: class c at row 32*(c%4) of bank c//4
        oin = r["o_in"].astype(np.float64)  # [B, 2, P, 512]
        for c in range(C):
            it[:, c] += oin[:, c // 4, 32 * (c % 4), :].sum(axis=1)
        otp = r["o_tp"].astype(np.float64)  # [B, P, 512]
        for c in range(4):
            tsm[:, c] += otp[:, 32 * c, :].sum(axis=1)
        tsm[:, 4:] += (r["o_ta"].astype(np.float64)
                       .reshape(B, P, 4, KCH).sum(axis=(1, 3)))

    dice = (2.0 * it + SMOOTH) / (ps + tsm + SMOOTH)
    loss = 1.0 - dice.mean(axis=1).mean(axis=0)
    return np.float32(loss)


# revision 25
# speedup vs baseline: 1.1050x; 1.1050x over previous
"""DiceLoss kernel for Trainium2 (8 NeuronCores, SPMD spatial sharding).

x: (4, 8, 64, 256, 256) f32 logits; target: (4, 1, 64, 256, 256) int labels.
loss = 1 - mean_b mean_c (2*inter[b,c]+1)/(psum[b,c]+tsum[b,c]+1)
  with p = softmax(x, axis=1) flattened over spatial S:
  inter[b,c] = sum_s p[b,c,s]*[t==c], psum = sum_s p, tsum = count(t==c).

Sharding: each core takes a contiguous 1/8 slice of S for all (b, c) and
produces partial sums of (psum, inter, tsum); the host reduces across
cores in f64 and applies the dice formula.

Per-core device pipeline, chunked [128 part, 8 class, 1024 free],
engine-balanced from measured per-op rates:
  SWDGE DMA: x f32->f16 cast, target int->f16 cast
  ScalarE: E = exp(x) fp16; lse = ln(denom); r = exp(-lse) fp16
  TensorE: denom via identity-matmul PSUM accumulation (contiguous rhs)
  VectorE: u_c = (E_c+0)*r scalar_tensor_tensor with accum_out -> psum
           m_c = (T==c) tensor_scalar (4x mode)
           w8 = u8*m8 one broadcast-free tensor_tensor (2x mode)
  TensorE: inter = ones32-matmuls over w8 slices, per-class [32-row, 512]
           PSUM regions via out base_partition (col-group placement)
  tsum: classes 0-3 on TensorE (same trick), 4-7 on ScalarE Copy+accum
"""

import sys
import time

import numpy as np

for _p in ("/opt/trn_rl_repo",):
    if _p not in sys.path:
        sys.path.insert(0, _p)

B = 4
C = 8
S = 64 * 256 * 256  # 4,194,304 spatial positions per (b, c)
NCORES = 8
SC = S // NCORES    # 524,288 positions per core per b
P = 128
F = 1024            # free positions per partition per chunk
CHUNK = P * F       # 131,072 positions per chunk
KCH = SC // CHUNK   # 4 chunks per b per core
SMOOTH = 1.0

PROFILE = False
RUN_KWARGS = {}
LAST = {}

_cache = {}

# walrus ldw-opt elides redundant LDWEIGHTS for repeated stationaries
# (concourse pins it off; our matmul streams reuse one stationary per pass)
# (walrus rejects bacc's explicit InstLdweights under ldw-opt; keep False)
ENABLE_LDW_OPT = False


def _patch_ldw_opt():
    import concourse.bass_utils as bu

    orig = bu.run_command

    def patched(argv, **kw):
        argv = ["--enable-ldw-opt=true" if a == "--enable-ldw-opt=false"
                else a for a in argv]
        return orig(argv, **kw)

    bu.run_command = patched
    return lambda: setattr(bu, "run_command", orig)


def _pin_act_tables():
    """Make natural_log_exp_and_others the only table providing Exp/Ln so
    the table-load pass emits one load instead of thrashing between the
    exp-only and ln-only sets. List positions are preserved (walrus maps
    sets by index)."""
    import concourse.bacc as bacc_mod
    from concourse import mybir

    orig = bacc_mod.get_activation_tables

    def patched(arch):
        tables = dict(orig(arch))
        exp = mybir.ActivationFunctionType.Exp
        ln = mybir.ActivationFunctionType.Ln
        for name, funcs in tables.items():
            if name != "natural_log_exp_and_others" and (
                exp in funcs or ln in funcs
            ):
                tables[name] = funcs - {exp, ln}
        return tables

    bacc_mod.get_activation_tables = patched
    return lambda: setattr(bacc_mod, "get_activation_tables", orig)


def _build(tgt_words):
    """Build + compile the Bass program. tgt_words=2 for int64 targets
    (int32 lo/hi pairs), 1 for int32 targets."""
    import concourse.bacc as bacc
    import concourse.tile as tile
    from concourse import mybir

    f32 = mybir.dt.float32
    f16 = mybir.dt.float16
    i32 = mybir.dt.int32
    Alu = mybir.AluOpType
    Act = mybir.ActivationFunctionType

    restore = _pin_act_tables()
    try:
        nc = bacc.Bacc("TRN2", target_bir_lowering=False)

        x_in = nc.dram_tensor("x", [B, C, SC], f32, kind="ExternalInput")
        if tgt_words == 2:
            t_in = nc.dram_tensor("t", [B, SC, 2], i32, kind="ExternalInput")
        else:
            t_in = nc.dram_tensor("t", [B, SC], i32, kind="ExternalInput")
        # psum partials: 2 PSUM banks, class c at row 32*(c%4) of bank c//4
        o_ps = nc.dram_tensor("o_ps", [B, 2, P, 512], f32,
                              kind="ExternalOutput")
        # inter partials: 2 PSUM banks, class c at row 32*(c%4) of bank c//4
        o_in = nc.dram_tensor("o_in", [B, 2, P, 512], f32,
                              kind="ExternalOutput")
        # tsum partials: classes 0-3 PE bank rows; 4-7 ScalarE accum slots
        o_tp = nc.dram_tensor("o_tp", [B, P, 512], f32, kind="ExternalOutput")
        o_ta = nc.dram_tensor("o_ta", [B, P, 4 * KCH], f32,
                              kind="ExternalOutput")

        xv = x_in[:].rearrange("b c (k p f) -> b k p c f", p=P, f=F)
        if tgt_words == 2:
            tv = t_in[:].rearrange("b (k p f) w -> b k p f w", p=P, f=F)
        else:
            tv = t_in[:].rearrange("b (k p f) -> b k p f", p=P, f=F)

        with tile.TileContext(nc) as tc:
            with (
                tc.tile_pool(name="const", bufs=1) as cpool,
                tc.tile_pool(name="xbuf", bufs=2) as xpool,
                tc.tile_pool(name="ebuf", bufs=2) as epool,
                tc.tile_pool(name="ubuf", bufs=2) as upool,
                tc.tile_pool(name="mbuf", bufs=2) as mpool,
                tc.tile_pool(name="wbuf", bufs=2) as wpool,
                tc.tile_pool(name="small", bufs=2) as spool,
                tc.tile_pool(name="stats", bufs=2) as stpool,
                tc.tile_pool(name="psum", bufs=1, space="PSUM") as ppool,
                tc.tile_pool(name="acc", bufs=1, space="PSUM") as apool,
            ):
                ident_i = cpool.tile([P, P], i32)
                nc.gpsimd.iota(ident_i[:], [[1, P]], channel_multiplier=-1)
                ident = cpool.tile([P, P], f16)
                nc.vector.tensor_scalar(
                    out=ident[:], in0=ident_i[:], scalar1=0, scalar2=None,
                    op0=Alu.is_equal,
                )
                ones32 = cpool.tile([P, 32], f16)
                nc.vector.memset(ones32[:], 1.0)

                for b in range(B):
                    st_ta = stpool.tile([P, 4 * KCH], f32, tag="st_ta")
                    acc_in0 = apool.tile([P, 512], f32, tag="acc_in0")
                    acc_in1 = apool.tile([P, 512], f32, tag="acc_in1")
                    acc_in = [acc_in0, acc_in1]
                    acc_ps0 = apool.tile([P, 512], f32, tag="acc_ps0")
                    acc_ps1 = apool.tile([P, 512], f32, tag="acc_ps1")
                    acc_ps = [acc_ps0, acc_ps1]
                    acc_tp = apool.tile([P, 512], f32, tag="acc_tp")

                    for kk in range(KCH):
                        first = kk == 0
                        last = kk == KCH - 1

                        xt = xpool.tile([P, C, F], f16, tag="xt")
                        nc.gpsimd.dma_start(out=xt[:], in_=xv[b, kk])

                        t16 = spool.tile([P, F], f16, tag="t16")
                        if tgt_words == 2:
                            nc.gpsimd.dma_start(out=t16[:],
                                                in_=tv[b, kk, :, :, 0])
                        else:
                            nc.gpsimd.dma_start(out=t16[:], in_=tv[b, kk])

                        e16 = epool.tile([P, C, F], f16, tag="e16")
                        nc.scalar.activation(e16[:], xt[:], Act.Exp)

                        dps = ppool.tile([P, F], f32, tag="dps")
                        for h in range(F // 512):
                            sl = slice(h * 512, (h + 1) * 512)
                            for c in range(C):
                                nc.tensor.matmul(
                                    dps[:, sl], ident[:], e16[:, c, sl],
                                    start=(c == 0), stop=(c == C - 1),
                                )

                        lse = spool.tile([P, F], f32, tag="lse")
                        nc.scalar.activation(lse[:], dps[:], Act.Ln)
                        r16 = spool.tile([P, F], f16, tag="r16")
                        nc.scalar.activation(r16[:], lse[:], Act.Exp,
                                             scale=-1.0)

                        # u = E * r in one broadcast tensor_tensor (2x)
                        u8 = upool.tile([P, C, F], f16, tag="u8")
                        rb = r16[:, None, :].broadcast_to((P, C, F))
                        nc.vector.tensor_tensor(
                            out=u8[:], in0=e16[:], in1=rb, op=Alu.mult)
                        # masks (4x tensor_scalar)
                        m8 = mpool.tile([P, C, F], f16, tag="m8")
                        for c in range(C):
                            nc.vector.tensor_scalar(
                                out=m8[:, c], in0=t16[:], scalar1=float(c),
                                scalar2=None, op0=Alu.is_equal,
                            )
                        # w = u * m in one 8-class op (2x tensor_tensor)
                        w8 = wpool.tile([P, C, F], f16, tag="w8")
                        nc.vector.tensor_tensor(
                            out=w8[:], in0=u8[:], in1=m8[:], op=Alu.mult)

                        # psum-red: ones32 matmuls over u8
                        for c in range(C):
                            row = 32 * (c % 4)
                            tp = (0, row) if row == 96 else None
                            for h in range(F // 512):
                                sl = slice(h * 512, (h + 1) * 512)
                                nc.tensor.matmul(
                                    acc_ps[c // 4][row:row + 32, :],
                                    ones32[:], u8[:, c, sl],
                                    start=(first and h == 0),
                                    stop=(last and h == F // 512 - 1),
                                    tile_position=tp,
                                    skip_group_check=True,
                                )
                        # inter-red: ones32 matmuls, class c -> rows
                        # 32*(c%4).. of bank c//4
                        for c in range(C):
                            row = 32 * (c % 4)
                            tp = (0, row) if row == 96 else None
                            for h in range(F // 512):
                                sl = slice(h * 512, (h + 1) * 512)
                                nc.tensor.matmul(
                                    acc_in[c // 4][row:row + 32, :],
                                    ones32[:], w8[:, c, sl],
                                    start=(first and h == 0),
                                    stop=(last and h == F // 512 - 1),
                                    tile_position=tp,
                                    skip_group_check=True,
                                )
                        # tsum-red classes 0-3 on PE
                        for c in range(4):
                            row = 32 * c
                            tp = (0, row) if row == 96 else None
                            for h in range(F // 512):
                                sl = slice(h * 512, (h + 1) * 512)
                                nc.tensor.matmul(
                                    acc_tp[row:row + 32, :],
                                    ones32[:], m8[:, c, sl],
                                    start=(first and h == 0),
                                    stop=(last and h == F // 512 - 1),
                                    tile_position=tp,
                                    skip_group_check=True,
                                )
                        # tsum-red classes 4-7 on ScalarE (Copy + accum)
                        for c in range(4, C):
                            col = (c - 4) * KCH + kk
                            dump = spool.tile([P, F], f16, tag="dump")
                            nc.scalar.activation(
                                dump[:], m8[:, c], Act.Copy,
                                accum_out=st_ta[:, col:col + 1],
                            )

                    # flush per-b results
                    nc.sync.dma_start(out=o_ta[b], in_=st_ta[:])
                    for half in range(2):
                        stg_ps = spool.tile([P, 512], f32, tag="stg")
                        nc.vector.tensor_copy(stg_ps[:], acc_ps[half][:])
                        nc.sync.dma_start(out=o_ps[b, half], in_=stg_ps[:])
                    for half in range(2):
                        stg = spool.tile([P, 512], f32, tag="stg")
                        nc.vector.tensor_copy(stg[:], acc_in[half][:])
                        nc.sync.dma_start(out=o_in[b, half], in_=stg[:])
                    stg_tp = spool.tile([P, 512], f32, tag="stg")
                    nc.vector.tensor_copy(stg_tp[:], acc_tp[:])
                    nc.sync.dma_start(out=o_tp[b], in_=stg_tp[:])

        nc.compile()
    finally:
        restore()
    return nc


def kernel(x, target):
    x = np.asarray(x)
    target = np.asarray(target)
    assert x.shape == (B, C, 64, 256, 256) and x.dtype == np.float32
    tgt_words = 2 if target.dtype == np.int64 else 1

    if tgt_words not in _cache:
        _cache[tgt_words] = _build(tgt_words)
    nc = _cache[tgt_words]

    xr = x.reshape(B, C, S)
    tr = target.reshape(B, S)

    in_maps = []
    for i in range(NCORES):
        sl = slice(i * SC, (i + 1) * SC)
        xs = np.ascontiguousarray(xr[:, :, sl])
        ts = np.ascontiguousarray(tr[:, sl])
        if tgt_words == 2:
            ts = ts.view(np.int32).reshape(B, SC, 2)
        else:
            ts = ts.astype(np.int32, copy=False)
        in_maps.append({"x": xs, "t": ts})

    from concourse.bass_utils import run_bass_kernel_spmd

    restore_ldw = _patch_ldw_opt() if ENABLE_LDW_OPT else (lambda: None)
    t0 = time.perf_counter()
    try:
        res = run_bass_kernel_spmd(
            nc, in_maps, list(range(NCORES)), trace=PROFILE, **RUN_KWARGS,
        )
    finally:
        restore_ldw()
    t1 = time.perf_counter()
    LAST["wall_s"] = t1 - t0
    LAST["exec_time_ns"] = res.exec_time_ns
    LAST["mean_exec_time_ns"] = res.mean_exec_time_ns

    ps = np.zeros((B, C), np.float64)
    it = np.zeros((B, C), np.float64)
    tsm = np.zeros((B, C), np.float64)
    for r in res.results:
        ops_ = r["o_ps"].astype(np.float64)  # [B, 2, P, 512]
        for c in range(C):
            ps[:, c] += ops_[:, c // 4, 32 * (c % 4), :].sum(axis=1)
        # inter: class c at row 32*(c%4) of bank c//4
        oin = r["o_in"].astype(np.float64)  # [B, 2, P, 512]
        for c in range(C):
            it[:, c] += oin[:, c // 4, 32 * (c % 4), :].sum(axis=1)
        otp = r["o_tp"].astype(np.float64)  # [B, P, 512]
        for c in range(4):
            tsm[:, c] += otp[:, 32 * c, :].sum(axis=1)
        tsm[:, 4:] += (r["o_ta"].astype(np.float64)
                       .reshape(B, P, 4, KCH).sum(axis=(1, 3)))

    dice = (2.0 * it + SMOOTH) / (ps + tsm + SMOOTH)
    loss = 1.0 - dice.mean(axis=1).mean(axis=0)
    return np.float32(loss)
